# revision 37
# baseline (speedup 1.0000x reference)
"""Trainium2 Bass kernel for nn_AdaptiveSynapticDelayAttention (B=8,S=2048,E=768,H=1).

Math notes
----------
* ``mscores`` in the reference is constant along the softmax (key) axis and
  softmax is shift-invariant, so the whole membrane-potential branch cannot
  change the output.  That removes the only cross-batch coupling -> pure
  data-parallel over batch: one batch element per NeuronCore, no collectives.
* With H=1 the attention collapses algebraically:
      scores = x @ (Wq^T Wk / sqrt(hd)) @ x^T =: x @ A @ x^T
      out    = softmax(scores) @ x @ (Wo Wv)^T + bo =: attn @ x @ Wov^T + bo
  A and Wov are weight-weight products computed (f32) on the host.
* The delay gather is a pure per-column shift applied host-side while packing.
* fp8 (e4m3, DoubleRow perf mode = 256-deep contraction at 2x rate) is used
  for the two S^2 matmul stages, which tolerate operand quantization:
    - scores^T = t1q^T @ x8 where t1 = (1024*A) @ x^T is produced in bf16
      (clean) and stored as fp8 (values ~N(0,11), comfortably inside e4m3
      range); Exp uses scale=1/1024 to undo the host pre-scale.
    - numerator: attn is fp8-quantized as a *deviation from its per-query
      mean*: e~ = exp - D/S (D = softmax denominator).  Then
        out_pre = (e~ @ x)/D + colsum(x)/S
      and the exact colsum term is folded into the output-projection bias on
      the host: bias' = bo + Wov @ mean_keys(x).  Centering shrinks the
      quantized magnitudes ~3.4x, so fp8 noise rides on the deviation only.
  t1-production and the output projection stay bf16 (fp8 there pushes the
  measured rel-err past the 2e-2 gate; simulated 1.81e-2 for this split).
* softmax without max-subtraction: logits ~N(0,0.3), exp() far from overflow.

Per-core schedule (one batch element)
-------------------------------------
  Software-pipelined across 512-wide q-tiles so the PE never idles waiting
  for the softmax denominator: after score-sweep(q), the first kc-tiles of
  sweep(q+1) run while ACT/DVE compute D(q), 1/D(q) and the 16 centered fp8
  e~ tiles; then numerator(q) (fp8), out-proj(q) (bf16), rest of sweep(q+1).
  t1 production (bf16, 36 matmuls of N=512) interleaves with sweep(0).
"""

import math
from contextlib import ExitStack

import numpy as np
import ml_dtypes

import concourse.bass as bass
import concourse.tile as tile
from concourse import bacc, mybir
from concourse.bass_utils import run_bass_kernel_spmd


def _install_ntff_hook():
    """The image's ``antenv`` lacks ``axon_hooks``, so the boot-time NTFF
    profile-hook registration degraded silently and trace=True would be
    skipped.  Recreate the module + hook here; degrade silently on any
    failure (tracing is optional, correctness never depends on it)."""
    try:
        import sys
        import types

        import antenv

        if hasattr(antenv, "axon_hooks"):
            return
        mod = types.ModuleType("antenv.axon_hooks")
        state = {"hook": None}
        mod.set_axon_ntff_profile_hook = lambda h: state.__setitem__("hook", h)
        mod.get_axon_ntff_profile_hook = lambda: state["hook"]
        sys.modules["antenv.axon_hooks"] = mod
        antenv.axon_hooks = mod
        from trn_agent_boot.trn_boot import _ntff_profile_via_ctypes

        mod.set_axon_ntff_profile_hook(
            _ntff_profile_via_ctypes("/opt/axon/libaxon_pjrt.so"))
    except Exception:
        pass


_install_ntff_hook()

BF16 = mybir.dt.bfloat16
F32 = mybir.dt.float32
FP8 = mybir.dt.float8e4
DR = mybir.MatmulPerfMode.DoubleRow

B, S, E = 8, 2048, 768
P = 128
EC = E // P          # 6 embed chunks
NPR = EC // 2        # 3 embed chunk-pairs (DoubleRow)
KC = S // P          # 16 key chunks
QT = 512             # q-tile width
NQT = S // QT        # 4 q tiles
TT = 512             # t-tile width for t1 production
ASC = 1024.0         # host pre-scale on A so fp8-stored t1 ~N(0,11)

# set by test.py to capture a profiled run
TRACE = False
LAST_RESULT = None

_BUILD_CACHE: dict = {}


def _build():
    nc = bacc.Bacc("TRN2", target_bir_lowering=False, debug=False, num_devices=8)

    # Host-packed blobs (partition-major, chunk-minor) so every load is one
    # contiguous multi-KB-per-partition 2-D DMA.
    #   xtb: bf16 x^T, 4 blobs of [128, 6x512] (t1-production moving operand)
    #   xt8: fp8 same layout (scores moving operand; ec-pairs adjacent)
    #   xn8: fp8 x-normal [128, 16x768] (numerator stationary; kc-pairs)
    #   at : bf16 (1024*A)^T contraction chunks, column-half blobs
    #   wov: bf16 Wov^T contraction chunks
    #   bo : f32 per-batch bias' = bo + Wov @ mean_keys(x)
    xtb_ext = nc.dram_tensor("xtb", [P, NQT * EC * QT], BF16, kind="ExternalInput")
    xt8_ext = nc.dram_tensor("xt8", [P, NQT * EC * QT], FP8, kind="ExternalInput")
    xn8_ext = nc.dram_tensor("xn8", [P, KC * E], FP8, kind="ExternalInput")
    at_ext = nc.dram_tensor("at", [P, EC * E], BF16, kind="ExternalInput")
    wov_ext = nc.dram_tensor("wov", [P, EC * E], BF16, kind="ExternalInput")
    bo_ext = nc.dram_tensor("bo", [E, 1], F32, kind="ExternalInput")
    out_ext = nc.dram_tensor("out", [E, S], BF16, kind="ExternalOutput")
    XB = EC * QT          # xtb/xt8 blob width (3072)
    AH = EC * (E // 2)    # at half-blob width (2304)

    with tile.TileContext(nc) as tc, ExitStack() as ctx:
        pers = ctx.enter_context(tc.tile_pool(name="pers", bufs=1))
        expp = ctx.enter_context(tc.tile_pool(name="expp", bufs=26))
        etp = ctx.enter_context(tc.tile_pool(name="etp", bufs=12))
        ptp = ctx.enter_context(tc.tile_pool(name="ptp", bufs=8))
        otp = ctx.enter_context(tc.tile_pool(name="otp", bufs=3))
        smallp = ctx.enter_context(tc.tile_pool(name="smallp", bufs=2))
        dtp = ctx.enter_context(tc.tile_pool(name="dtp", bufs=15))
        psp = ctx.enter_context(tc.tile_pool(name="psp", bufs=1, space="PSUM"))

        # ---- persistent SBUF tensors ----
        xtb_all = pers.tile([P, NQT * XB], BF16, name="xtb_all", tag="xtb_all")
        xt8_all = pers.tile([P, NQT * EC, QT], FP8, name="xt8_all", tag="xt8_all")
        xn8_all = pers.tile([P, KC, E], FP8, name="xn8_all", tag="xn8_all")
        at_all = pers.tile([P, EC * E], BF16, name="at_all", tag="at_all")
        wov_all = pers.tile([P, EC * E], BF16, name="wov_all", tag="wov_all")
        # t1 stored fp8 as ec-chunk pairs: [128, 2, 2048] x 3 (scores lhsT)
        t1p = [pers.tile([P, 2, S], FP8, name=f"t1p{c}", tag=f"t1p{c}")
               for c in range(NPR)]

        def xtb_ap(c, t0, w):
            """bf16 x^T chunk c, time-cols [t0, t0+w) within one 512 blob."""
            b_, o = t0 // QT, t0 % QT
            assert o + w <= QT
            base = b_ * XB + c * QT + o
            return xtb_all[:, base:base + w]

        def at_ap(jc, ic):
            """(1024A)^T chunk jc, output-block ic (half-blob layout)."""
            h, i = divmod(ic, EC // 2)
            base = h * AH + jc * (E // 2) + i * P
            return at_all[:, base:base + P]

        bo_sb = pers.tile([P, EC], F32, name="bo_sb", tag="bo_sb")
        # ones matrix pre-scaled by 1/S: the denominator matmul then directly
        # yields psS = D/S (the centering constant), the reciprocal yields
        # S/D, and the stray factor S in pt is folded into the out-proj
        # activation's free `scale` parameter (all powers of 2 -> exact).
        ones_bf = pers.tile([P, P], BF16, name="ones_bf", tag="ones_bf")
        nc.vector.memset(ones_bf[:, :], 1.0 / float(S))
        scr = pers.tile([P, QT], BF16, name="scr", tag="scr")
        nc.vector.memset(scr[:, :], 0.5)

        # ---- PE warm-up (HAM clock gate): burn the input-load window on
        # dummy matmuls so the first real matmul runs at 2.4 GHz ----
        for w in range(17):
            pw = psp.tile([P, QT], F32, name="po", tag="po", bufs=2)
            nc.tensor.matmul(pw[:, 0:QT // 2], lhsT=ones_bf[:, :],
                             rhs=scr[:, 0:QT // 2], start=True, stop=True)

        # ---- loads: fine-grained round-robin waves, the t1 critical path
        # (at half 0 + xtb blob 0) spread over 5 engine queues so the first
        # real matmul's data lands as early as possible; later waves use the
        # 3 rings that stay free during compute ----
        engs = (nc.sync, nc.scalar, nc.gpsimd)
        crit = engs
        rr = 0

        def ld(dst, src, a, b_, first=False):
            nonlocal rr
            pool = crit if first else engs
            pool[rr % len(pool)].dma_start(out=dst[:, a:b_], in_=src[:, a:b_])
            rr += 1

        def ld8(c):
            nonlocal rr
            engs[rr % 3].dma_start(out=xt8_all[:, c, :],
                                   in_=xt8_ext[:, c * QT:(c + 1) * QT])
            rr += 1

        # critical first wave: the whole at blob then xtb blob0, each as 3
        # large contiguous per-ring transfers (both SBUF regions are
        # contiguous), so the aggregate queue bandwidth is spent on exactly
        # the bytes the t1 chains need and at never gates a chain again.
        for k in range(3):
            ld(at_all, at_ext, k * (2 * AH // 3), (k + 1) * (2 * AH // 3),
               first=True)
        for k in range(3):
            ld(xtb_all, xtb_ext, k * (XB // 3), (k + 1) * (XB // 3), first=True)
        for c in range(EC):      # xt8 blob0 (needed at sweep(0) kc0)
            ld8(c)
        for c in range(EC):      # xtb blob1
            ld(xtb_all, xtb_ext, XB + c * QT, XB + (c + 1) * QT)
        for c in range(EC):      # xtb blob2
            ld(xtb_all, xtb_ext, 2 * XB + c * QT, 2 * XB + (c + 1) * QT)
        for c in range(EC):      # xtb blob3
            ld(xtb_all, xtb_ext, 3 * XB + c * QT, 3 * XB + (c + 1) * QT)
        for c in range(EC, 2 * EC):   # xt8 blob1
            ld8(c)
        for t in range(KC):      # xn8
            nc.gpsimd.dma_start(out=xn8_all[:, t, :],
                                in_=xn8_ext[:, t * E:(t + 1) * E])
        for c in range(2 * EC, 4 * EC):   # xt8 blobs 2-3
            ld8(c)
        for c in range(EC):
            nc.gpsimd.dma_start(out=wov_all[:, c * E:(c + 1) * E],
                                in_=wov_ext[:, c * E:(c + 1) * E])
        for c in range(EC):
            nc.gpsimd.dma_start(out=bo_sb[:, c:c + 1], in_=bo_ext[c * P:(c + 1) * P, :])

        # ---- t1 = (1024A) @ x^T in bf16, stored fp8 into pair tiles ----
        def t1_piece(c0, w):
            for ic in range(EC):
                ps = psp.tile([P, TT], F32, name="mmps", tag="mmps", bufs=2)
                for jc in range(EC):
                    nc.tensor.matmul(
                        ps[:, 0:w],
                        lhsT=at_ap(jc, ic),
                        rhs=xtb_ap(jc, c0, w),
                        start=(jc == 0),
                        stop=(jc == EC - 1),
                    )
                nc.vector.tensor_copy(
                    out=t1p[ic // 2][:, ic % 2, c0:c0 + w], in_=ps[:, 0:w])

        exps = {}
        roots = {}

        def sweep_gen(q):
            """Score sweep for q-tile q: 16 kc steps, fp8 DoubleRow matmuls,
            Exp to bf16, bf16 add-tree for the denominator.  16 yields."""
            exp_tiles = []
            tree = []

            def tree_add(a, b_):
                o = dtp.tile([P, QT], BF16, name="dt", tag="dt", bufs=15)
                nc.vector.tensor_add(o[:, :], a[:, :], b_[:, :])
                return o

            if q == 0:
                t1_piece(0, TT // 2)
                t1_piece(TT // 2, TT // 2)
                t1_piece(TT, TT)
            for kc in range(KC):
                if q == 0 and kc in (4, 8):
                    t1_piece(2 * TT if kc == 4 else 3 * TT, TT)
                ps = psp.tile([P, QT], F32, name="scl", tag="scl", bufs=3)
                for pr in range(NPR):
                    nc.tensor.matmul(
                        ps[:, :],
                        lhsT=t1p[pr][:, :, kc * P:(kc + 1) * P],
                        rhs=xt8_all[:, EC * q + 2 * pr:EC * q + 2 * pr + 2, :],
                        start=(pr == 0),
                        stop=(pr == NPR - 1),
                        perf_mode=DR,
                    )
                e = expp.tile([P, QT], BF16, name="exp", tag="exp", bufs=26)
                nc.scalar.activation(
                    out=e[:, :], in_=ps[:, :],
                    func=mybir.ActivationFunctionType.Exp,
                    scale=1.0 / ASC,
                )
                exp_tiles.append(e)
                if kc % 2 == 1:
                    tree.append(tree_add(exp_tiles[kc - 1], e))
                if kc == KC - 1:
                    while len(tree) > 1:
                        tree = [tree_add(tree[2 * i], tree[2 * i + 1])
                                for i in range(len(tree) // 2)]
                    exps[q] = exp_tiles
                    roots[q] = tree[0]
                yield

        def consume(g, n):
            for _ in range(n):
                next(g, None)

        def emit_outproj(q, pt_tiles):
            """Output projection (bf16) for q-tile q.  pt carries a stray
            factor S (pt = pv * S/D); the activation scale 1/S removes it
            exactly (power of 2) before the bias is added."""
            q0 = q * QT
            oengs = ((nc.sync, nc.scalar) if q < NQT - 1 else
                     (nc.sync, nc.scalar, nc.gpsimd))
            last = NQT * EC - 1
            for ic in range(EC):
                if q * EC + ic < last:
                    po = psp.tile([P, QT], F32, name="po", tag="po", bufs=2)
                    for jc in range(EC):
                        nc.tensor.matmul(
                            po[:, :],
                            lhsT=wov_all[:, jc * E + ic * P:jc * E + (ic + 1) * P],
                            rhs=pt_tiles[jc][:, :],
                            start=(jc == 0),
                            stop=(jc == EC - 1),
                        )
                    ot = otp.tile([P, QT], BF16, name="ot", tag="ot", bufs=3)
                    nc.scalar.activation(
                        out=ot[:, :], in_=po[:, :],
                        func=mybir.ActivationFunctionType.Identity,
                        bias=bo_sb[:, ic:ic + 1], scale=1.0 / float(S))
                    oengs[ic % len(oengs)].dma_start(
                        out=out_ext[ic * P:(ic + 1) * P, q0:q0 + QT], in_=ot[:, :])
                else:
                    # very last output block: two half-width psum groups so
                    # the final ACT+store is half-sized and overlaps the
                    # second group's matmuls, shrinking the kernel tail.
                    for h, tag in ((0, "den"), (1, "po")):
                        poh = psp.tile([P, QT], F32, name="poh", tag=tag,
                                       bufs=1 if tag == "den" else 2)
                        for jc in range(EC):
                            nc.tensor.matmul(
                                poh[:, 0:QT // 2],
                                lhsT=wov_all[:, jc * E + ic * P:jc * E + (ic + 1) * P],
                                rhs=pt_tiles[jc][:, h * (QT // 2):(h + 1) * (QT // 2)],
                                start=(jc == 0),
                                stop=(jc == EC - 1),
                            )
                        ot = otp.tile([P, QT], BF16, name="ot", tag="ot", bufs=3)
                        nc.scalar.activation(
                            out=ot[:, 0:QT // 2], in_=poh[:, 0:QT // 2],
                            func=mybir.ActivationFunctionType.Identity,
                            bias=bo_sb[:, ic:ic + 1], scale=1.0 / float(S))
                        oengs[h % 3].dma_start(
                            out=out_ext[ic * P:(ic + 1) * P,
                                        q0 + h * (QT // 2):q0 + (h + 1) * (QT // 2)],
                            in_=ot[:, 0:QT // 2])

        gens = [sweep_gen(q) for q in range(NQT)]
        consume(gens[0], KC)

        held = {}  # q -> pt_tiles: out-proj deferred to hide subs(q+1)

        for q in range(NQT):
            q0 = q * QT
            nq = q + 1
            if nq < NQT:
                consume(gens[nq], 3)
            # denominator: ones(1/S)-matmul -> psS = D/S broadcast over all
            # partitions.  The 16 center-and-quantize subs read psS (f32
            # psum) directly -- every variant of this op is ~600ns (the fp8
            # write dominates), so pair 0 is emitted first and the
            # reciprocal right after it, letting the numerator start
            # ~1.2us after the ones-matmul lands.
            psS = psp.tile([P, QT], F32, name="den", tag="den", bufs=1)
            nc.tensor.matmul(psS[:, :], lhsT=ones_bf[:, :], rhs=roots[q][:, :],
                             start=True, stop=True)
            et_tiles = [etp.tile([P, 2, QT], FP8, name="et", tag="et", bufs=12)
                        for _ in range(KC // 2)]

            def sub(eng, pr, i):
                eng.scalar_tensor_tensor(
                    out=et_tiles[pr][:, i, :], in0=psS[:, :], scalar=-1.0,
                    in1=exps[q][2 * pr + i][:, :],
                    op0=mybir.AluOpType.mult, op1=mybir.AluOpType.add)

            for pr in range(2):
                for i in range(2):
                    sub(nc.vector, pr, i)
            bc_sb = smallp.tile([P, QT], F32, name="bc_sb", tag="bc_sb", bufs=2)
            nc.vector.reciprocal_approx_fast(bc_sb[:, :], psS[:, :])
            for pr in range(2, KC // 2):
                for i in range(2):
                    sub(nc.vector, pr, i)
            del exps[q]
            if (q - 1) in held:
                # every out-proj is deferred one iteration: the PE chews
                # q-1's out-proj exactly while the DVE produces this
                # q-tile's 16 e~ tiles, so the numerator never starves on
                # the fp8-write-bound centering subs.
                emit_outproj(q - 1, held.pop(q - 1))
            if nq < NQT:
                consume(gens[nq], 3)
            # numerator (fp8 DoubleRow over kc-pairs): pv[j,q] = e~^T-contract
            pt_tiles = []
            for jc in range(EC):
                pv = psp.tile([P, QT], F32, name="pav", tag="mmps", bufs=2)
                for pr in range(KC // 2):
                    nc.tensor.matmul(
                        pv[:, :],
                        lhsT=xn8_all[:, 2 * pr:2 * pr + 2, jc * P:(jc + 1) * P],
                        rhs=et_tiles[pr][:, :, :],
                        start=(pr == 0),
                        stop=(pr == KC // 2 - 1),
                        perf_mode=DR,
                    )
                pt = ptp.tile([P, QT], BF16, name="pt", tag="pt", bufs=20)
                nc.vector.tensor_mul(pt[:, :], pv[:, :], bc_sb[:, :])
                pt_tiles.append(pt)
            if nq < NQT:
                consume(gens[nq], 10)
            if q < NQT - 1:
                held[q] = pt_tiles      # deferred into the next iteration
            else:
                emit_outproj(q, pt_tiles)

    nc.compile()
    return nc


def _delayed(spikes, dw):
    """delayed[b,t,n] = spikes[b, t-d[n], n] (0 for t<d[n]) — a pure
    per-column shift, applied host-side while packing layouts."""
    b, s, e = spikes.shape
    out = np.zeros_like(spikes)
    for d in np.unique(dw):
        cols = np.nonzero(dw == d)[0]
        d = int(d)
        if d <= 0:
            out[:, :, cols] = spikes[:, :, cols] if d == 0 else 0
        elif d < s:
            out[:, d:, cols] = spikes[:, :s - d, cols]
    return out


def kernel(**inputs) -> np.ndarray:
    global LAST_RESULT
    spikes = np.asarray(inputs["spikes"], dtype=np.float32)
    dw = np.asarray(inputs["delay_weights"]).reshape(-1).astype(np.int64)
    Wq = np.asarray(inputs["Wq"], dtype=np.float32)
    Wk = np.asarray(inputs["Wk"], dtype=np.float32)
    Wv = np.asarray(inputs["Wv"], dtype=np.float32)
    Wo = np.asarray(inputs["Wo"], dtype=np.float32)
    bo = np.asarray(inputs["bo"], dtype=np.float32)

    if "dev" not in _BUILD_CACHE:
        _BUILD_CACHE["dev"] = _build()
    nc = _BUILD_CACHE["dev"]

    bf = ml_dtypes.bfloat16
    f8 = ml_dtypes.float8_e4m3

    def q8(x):
        return np.clip(x, -240.0, 240.0).astype(f8)

    P_, EC_, KC_, QT_ = 128, E // 128, S // 128, 512
    # weight-weight fusions (f32 on host): A = 1024 * Wq^T Wk / sqrt(hd)
    # (power-of-2 pre-scale so the fp8-stored t1 sits in e4m3's sweet spot),
    # Wov = Wo Wv.
    A = (Wq.T @ Wk) * np.float32(ASC / math.sqrt(E))
    Wov = Wo @ Wv
    at3 = A.T.astype(bf).reshape(EC_, P_, E).transpose(1, 0, 2)     # [128,6,768]
    atP = np.ascontiguousarray(np.concatenate(
        [at3[:, :, 0:E // 2], at3[:, :, E // 2:E]], axis=1).reshape(P_, EC_ * E))
    wovP = np.ascontiguousarray(
        Wov.T.astype(bf).reshape(EC_, P_, E).transpose(1, 0, 2)
        .reshape(P_, EC_ * E))

    delayed = _delayed(spikes, dw)
    in_maps = []
    for b in range(B):
        xt3 = delayed[b].T.reshape(EC_, P_, S)                      # [6,128,2048]
        xt4 = xt3.reshape(EC_, P_, S // QT_, QT_).transpose(1, 2, 0, 3)
        xtbP = np.ascontiguousarray(xt4.astype(bf).reshape(P_, EC_ * S))
        xt8P = np.ascontiguousarray(q8(xt4).reshape(P_, EC_ * S))
        xn8P = np.ascontiguousarray(
            q8(delayed[b].reshape(KC_, P_, E).transpose(1, 0, 2))
            .reshape(P_, KC_ * E))
        # bias' = bo + Wov @ mean_keys(x): the exact mean-attention term of
        # the centered-numerator decomposition, folded into the bias.
        corr = (Wov @ (delayed[b].sum(axis=0) / np.float32(S)) + bo)
        in_maps.append({"xtb": xtbP, "xt8": xt8P, "xn8": xn8P, "at": atP,
                        "wov": wovP, "bo": np.ascontiguousarray(
                            corr.reshape(E, 1).astype(np.float32))})

    LAST_RESULT = run_bass_kernel_spmd(
        nc, in_maps, core_ids=list(range(B)), trace=TRACE,
    )
    out = np.stack([LAST_RESULT.results[b]["out"].astype(np.float32).T
                    for b in range(B)])
    return np.ascontiguousarray(out)
